# revision 13
# baseline (speedup 1.0000x reference)
"""Multi-head causal attention (dense transformer block) on 8 Trainium2 cores.

Sharding: 2-way data parallel over batch x 4-way tensor parallel over heads.
Core c handles batch c//4 and heads 4*(c%4) .. 4*(c%4)+3.

Per-core pipeline (all activation layouts chosen so no on-device transposes
are needed; host pre-transposes x and the weight shards once):
  1. QT/KT [hd, t] (bf16) and V [t, hd] (bf16) projections from xT [d, t]
     at fp32r. Weight tiles arrive pre-packed so one DMA carries two
     d-subtiles. Weight DMAs issue on the DVE queue, x tiles on SP, PSUM
     evictions on ACT, so no queue serializes another phase's traffic.
  2. Attention per (head, q-chunk) in bf16 with scores computed transposed
     (S^T [k, q]) at exact causal widths (bf16 matmul has no minimum
     moving-dim for full rate). exp on ACT without max-subtraction; causal
     masking via a resident upper-triangle 0/1 mask multiplied on DVE
     (keeps the Pool queue free for collectives). Softmax denominators are
     accumulated across k-tiles on DVE (bf16 2x mode) and partition-summed
     + broadcast in a single ones[128,128] matmul per (head, chunk) —
     removing the per-tile denominator matmuls from the PE critical path.
  3. Output projection at fp32r from resident weights. Its matmuls are
     interleaved into the NEXT chunk's attention steps so the PE never
     stalls while ACT works through exp.
  4. Chunked bf16 ReduceScatter over the 4 cores sharing a batch; host
     casts to f32 and concatenates the row shards.
"""

import os
import sys

sys.path.insert(0, "/opt/trn_rl_repo")

import numpy as np

N_CORES = 8
B = 2
T = 2048          # sequence length
D = 2048          # model dim
P = 128           # partitions
HD = 128          # head dim
NHG = 4           # head-groups (cores per batch)
HPC = 4           # heads per core
F = HPC * HD      # 512 per-core q/k/v feature width
TC = 512          # token chunk (matmul free dim)
NTC = T // TC     # 4 token chunks
ND = D // P       # 16 d-subtiles
NJ = ND // 2      # 8 packed weight tiles (2 d-subtiles each)
SCALE = float(HD) ** -0.5

_CACHE = {}


def _build(mm_dtype_name: str, reps: int = 1, with_rs: bool = True):
    import concourse.bacc as bacc
    import concourse.mybir as mybir
    import concourse.tile as tile

    dt = mybir.dt
    f32 = dt.float32
    bf = dt.bfloat16
    md = getattr(dt, mm_dtype_name)  # dtype of fp32-path PE-input tiles

    nc = bacc.Bacc(
        "TRN2", target_bir_lowering=False, debug=False, num_devices=N_CORES
    )

    xT = nc.dram_tensor("xT", [D, T], md, kind="ExternalInput")
    # packed: [j*128+p, sub*512+f] = W^T[(2j+sub)*128+p, f]
    wqP = nc.dram_tensor("wqP", [D // 2, 2 * F], md, kind="ExternalInput")
    wkP = nc.dram_tensor("wkP", [D // 2, 2 * F], md, kind="ExternalInput")
    wvP = nc.dram_tensor("wvP", [D // 2, 2 * F], md, kind="ExternalInput")
    woT = nc.dram_tensor("woT", [F, D], md, kind="ExternalInput")
    out = nc.dram_tensor("out", [T // NHG, D], bf, kind="ExternalOutput")

    with nc.allow_low_precision(reason="bf16 attention internals"), \
         tile.TileContext(nc) as tc:
        with (
            tc.tile_pool(name="const", bufs=1) as const,
            tc.tile_pool(name="resident", bufs=1) as res_pool,
            tc.tile_pool(name="dram", bufs=1, space="DRAM") as dram,
        ):
            ones128 = const.tile([P, P], bf)
            nc.vector.memset(ones128[:], 1.0)
            # upper-triangle keep-mask for the diagonal score blocks:
            # tri[p, c] = 1 if c >= p else 0  (S^T layout: p=k, c=q)
            tri = const.tile([P, P], bf)
            nc.vector.memset(tri[:], 1.0)
            nc.gpsimd.affine_select(
                tri[:], tri[:],
                pattern=[[1, P]],
                compare_op=mybir.AluOpType.is_ge,
                fill=0.0,
                base=0,
                channel_multiplier=-1,
            )

            # ---- resident activation buffers ----
            QT = [res_pool.tile([P, T], bf, name=f"QT{h}") for h in range(HPC)]
            KT = [res_pool.tile([P, T], bf, name=f"KT{h}") for h in range(HPC)]
            V = [res_pool.tile([P, F], bf, name=f"V{i}") for i in range(T // P)]
            WO = [
                [res_pool.tile([P, 2 * TC], md, name=f"WO{ci}_{etp}")
                 for etp in range(NTC // 2)]
                for ci in range(HPC)
            ]

            bounce = [dram.tile([TC, D], bf, name=f"bounce{qt}")
                      for qt in range(NTC - 1)]
            bounce += [dram.tile([TC // 2, D], bf, name=f"bounce3{hf}")
                       for hf in range(2)]
            rs_out = [dram.tile([TC // NHG, D], bf, name=f"rs_out{qt}")
                      for qt in range(NTC - 1)]
            rs_out += [dram.tile([TC // 2 // NHG, D], bf, name=f"rs_out3{hf}")
                       for hf in range(2)]

            for rep in range(reps):
                _build_body(nc, tc, mybir, md, bf, f32, rep,
                            xT, wqP, wkP, wvP, woT, out,
                            ones128, tri, QT, KT, V, WO,
                            bounce, rs_out, with_rs)

    nc.compile()
    return nc


def _build_body(nc, tc, mybir, md, bf, f32, rep,
                xT, wqP, wkP, wvP, woT, out,
                ones128, tri, QT, KT, V, WO,
                bounce, rs_out, with_rs=True):
    # ---- phase 0: resident output-projection weights (Pool/SWDGE queue,
    # idle during phase 1, so these don't delay the first wq tiles) ----
    if rep == 0:
        for ci in range(HPC):
            for etp in range(NTC // 2):
                nc.gpsimd.dma_start(
                    WO[ci][etp][:],
                    woT.ap()[ci * P:(ci + 1) * P,
                             etp * 2 * TC:(etp + 1) * 2 * TC],
                )

    # ---- phase 1: projections ----
    # Two supersteps of 1024 tokens; each loads the packed q/k/v weights
    # once (24 MB instead of 48 MB of weight traffic per pass over x).
    TG = 2 * TC
    with tc.tile_pool(name=f"psum1_{rep}", bufs=1, space="PSUM") as psum1, \
         tc.tile_pool(name=f"xw_{rep}", bufs=3) as xw_pool:
        for tg in range(T // TG):
            xts = []
            for di in range(ND):
                xt = xw_pool.tile(
                    [P, TG], md, name=f"xt_{rep}_{tg}_{di}", tag="xt",
                    bufs=ND + 2,
                )
                nc.sync.dma_start(
                    xt[:],
                    xT.ap()[di * P:(di + 1) * P, tg * TG:(tg + 1) * TG],
                )
                xts.append(xt)
            wts = {}
            for wname, wP in (("q", wqP), ("k", wkP), ("v", wvP)):
                for j in range(NJ):
                    wt = xw_pool.tile(
                        [P, 2 * F], md, name=f"w{wname}_{rep}_{tg}_{j}",
                        tag="wt", bufs=6,
                    )
                    nc.scalar.dma_start(wt[:], wP.ap()[j * P:(j + 1) * P, :])
                    wts[wname, j] = wt
            for wname, dest in (("q", QT), ("k", KT)):
                pss = [
                    psum1.tile(
                        [P, TC], f32, name=f"ps_{wname}{h}{th}_{rep}_{tg}",
                        tag="pq", bufs=8,
                    )
                    for h in range(HPC) for th in range(2)
                ]
                for j in range(NJ):
                    wt = wts[wname, j]
                    for sub in range(2):
                        di = 2 * j + sub
                        for h in range(HPC):
                            for th in range(2):
                                nc.tensor.matmul(
                                    pss[2 * h + th][:],
                                    wt[:, sub * F + h * HD:
                                       sub * F + (h + 1) * HD],
                                    xts[di][:, th * TC:(th + 1) * TC],
                                    start=(di == 0),
                                    stop=(di == ND - 1),
                                )
                for h in range(HPC):
                    for th in range(2):
                        col = tg * TG + th * TC
                        nc.vector.tensor_copy(
                            dest[h][:, col:col + TC], pss[2 * h + th][:]
                        )
            pss = [
                psum1.tile(
                    [P, F], f32, name=f"ps_v{ts}_{rep}_{tg}", tag="pq", bufs=8
                )
                for ts in range(TG // P)
            ]
            for j in range(NJ):
                wt = wts["v", j]
                for sub in range(2):
                    di = 2 * j + sub
                    for ts in range(TG // P):
                        nc.tensor.matmul(
                            pss[ts][:],
                            xts[di][:, ts * P:(ts + 1) * P],
                            wt[:, sub * F:(sub + 1) * F],
                            start=(di == 0),
                            stop=(di == ND - 1),
                        )
            # V evictions on ACT (GPSIMD cannot read PSUM) so the DVE
            # queue is clear for the first attention masks/accumulates
            for ts in range(TG // P):
                nc.scalar.copy(V[tg * (TG // P) + ts][:], pss[ts][:])

    # ---- phases 2+3, software-pipelined per q chunk ----
    with tc.tile_pool(name=f"psum2_{rep}", bufs=1, space="PSUM") as psum2, \
         tc.tile_pool(name=f"work_{rep}", bufs=6) as work:

        def make_outproj(qt, outT):
            """Closures, one per (ts, etp): 8 fp32r matmuls + 2 evictions
            (split ACT/DVE) + the bounce-row DMA (SP)."""
            last = qt == NTC - 1
            closures = []

            def mk(ts, etp):
                def run():
                    fin = work.tile(
                        [P, 2 * TC], bf, name=f"fin{rep}_{qt}_{ts}_{etp}",
                        tag="fin", bufs=4,
                    )
                    # the last chunk's closures run after attention is done,
                    # so they may borrow the idle 'st' banks for deeper
                    # pipelining of the eviction chain
                    tag = "f" if (not last or (ts + etp) % 2 == 0) else "st"
                    psf = [
                        psum2.tile(
                            [P, TC], f32,
                            name=f"ps_f{rep}_{qt}_{ts}_{etp}_{ee}",
                            tag=tag, bufs=2,
                        )
                        for ee in range(2)
                    ]
                    for ci in range(HPC):
                        for ee in range(2):
                            nc.tensor.matmul(
                                psf[ee][:],
                                outT[ci][:, ts * P:(ts + 1) * P],
                                WO[ci][etp][:, ee * TC:(ee + 1) * TC],
                                start=(ci == 0),
                                stop=(ci == HPC - 1),
                            )
                    nc.scalar.copy(fin[:, 0:TC], psf[0][:])
                    nc.vector.tensor_copy(fin[:, TC:2 * TC], psf[1][:])
                    if last:
                        dst = bounce[NTC - 1 + ts // 2]
                        drow = (ts % 2) * P
                    else:
                        dst = bounce[qt]
                        drow = ts * P
                    nc.sync.dma_start(
                        dst[drow:drow + P,
                            etp * 2 * TC:(etp + 1) * 2 * TC],
                        fin[:],
                    )
                return run

            for ts in range(TC // P):
                for etp in range(NTC // 2):
                    closures.append(mk(ts, etp))
            return closures

        def finish_chunk(qt):
            """Post-outproj collective for chunk qt."""
            if qt < NTC - 1:
                if with_rs:
                    nc.gpsimd.collective_compute(
                        "ReduceScatter",
                        mybir.AluOpType.add,
                        replica_groups=[[0, 1, 2, 3], [4, 5, 6, 7]],
                        ins=[bounce[qt].opt()],
                        outs=[rs_out[qt].opt()],
                    )
                    nc.gpsimd.dma_start(
                        out.ap()[qt * (TC // NHG):(qt + 1) * (TC // NHG), :],
                        rs_out[qt][:],
                    )
                else:
                    nc.gpsimd.dma_start(
                        out.ap()[qt * (TC // NHG):(qt + 1) * (TC // NHG), :],
                        bounce[qt][0:TC // NHG, :],
                    )
            else:
                for hf in range(2):
                    rw = TC // 2 // NHG
                    base = qt * (TC // NHG) + hf * rw
                    if with_rs:
                        nc.gpsimd.collective_compute(
                            "ReduceScatter",
                            mybir.AluOpType.add,
                            replica_groups=[[0, 1, 2, 3], [4, 5, 6, 7]],
                            ins=[bounce[NTC - 1 + hf].opt()],
                            outs=[rs_out[NTC - 1 + hf].opt()],
                        )
                        nc.gpsimd.dma_start(
                            out.ap()[base:base + rw, :],
                            rs_out[NTC - 1 + hf][:],
                        )
                    else:
                        nc.gpsimd.dma_start(
                            out.ap()[base:base + rw, :],
                            bounce[NTC - 1 + hf][0:rw, :],
                        )

        pending = []      # rolling queue of outproj closures (FIFO)

        def drain(n):
            for _ in range(min(n, len(pending))):
                pending.pop(0)()

        for qt in range(NTC):
            outT = {}
            n_k = (qt + 1) * (TC // P)  # causal: k-subtiles needed
            diag0 = qt * (TC // P)
            korder = list(range(diag0, n_k)) + list(range(diag0))
            SKEW = 2
            for hp in (0, 2):  # head pairs, emission interleaved
                heads = (hp, hp + 1)
                ps_out = {
                    h: psum2.tile(
                        [P, TC], f32, name=f"ps_out{rep}_{qt}_{h}",
                        tag="out", bufs=2,
                    )
                    for h in heads
                }
                acc = {
                    h: work.tile(
                        [P, TC], bf, name=f"acc{rep}_{qt}_{h}", tag="acc",
                        bufs=4,
                    )
                    for h in heads
                }
                pts = {}

                def live0(kt):
                    # exact causal width: bf16 matmuls run full rate at any
                    # moving size, so no 256-wide clamp is needed
                    return max(0, kt - diag0) * P

                for step in range(n_k + SKEW):
                    if step < n_k:
                        kt = korder[step]
                        c0 = live0(kt)
                        for h in heads:
                            ps_st = psum2.tile(
                                [P, TC], f32,
                                name=f"ps_st{rep}_{qt}_{h}_{kt}",
                                tag="st", bufs=2,
                            )
                            nc.tensor.matmul(
                                ps_st[:, c0:],
                                KT[h][:, kt * P:(kt + 1) * P],
                                QT[h][:, qt * TC + c0:(qt + 1) * TC],
                                start=True,
                                stop=True,
                            )
                            pt = work.tile(
                                [P, TC], bf, name=f"pt{rep}_{qt}_{h}_{kt}",
                                tag="pt", bufs=8,
                            )
                            nc.scalar.activation(
                                pt[:, c0:], ps_st[:, c0:],
                                mybir.ActivationFunctionType.Exp,
                                scale=SCALE,
                            )
                            dj = kt - diag0
                            if dj >= 0:
                                # zero the strictly-upper part of the
                                # diagonal 128x128 block (q < k)
                                me = (dj + 1) * P
                                nc.vector.tensor_mul(
                                    pt[:, c0:me], pt[:, c0:me], tri[:]
                                )
                            if step == 0:
                                nc.vector.tensor_copy(acc[h][:], pt[:])
                            else:
                                nc.vector.tensor_add(
                                    acc[h][:, c0:], acc[h][:, c0:],
                                    pt[:, c0:],
                                )
                            pts[h, kt] = pt
                    if step >= SKEW:
                        idx = step - SKEW
                        k = korder[idx]
                        c0 = live0(k)
                        for h in heads:
                            nc.tensor.matmul(
                                ps_out[h][:, c0:],
                                V[k][:, h * HD:(h + 1) * HD],
                                pts[h, k][:, c0:],
                                start=(idx == 0),
                                stop=(idx == n_k - 1),
                            )
                    drain(1)
                for h in heads:
                    ps_bc = psum2.tile(
                        [P, TC], f32, name=f"ps_bc{rep}_{qt}_{h}", tag="aux",
                        bufs=2,
                    )
                    nc.tensor.matmul(
                        ps_bc[:], ones128[:], acc[h][:],
                        start=True, stop=True,
                    )
                    # evict AV to SBUF on ACT right away: frees the PSUM
                    # bank for the next chunk without waiting on the DVE
                    # reciprocal chain
                    av = work.tile([P, TC], f32, name=f"av{rep}_{qt}_{h}",
                                   tag="av", bufs=2)
                    nc.scalar.copy(av[:], ps_out[h][:])
                    rec = work.tile([P, TC], f32, name=f"rec{rep}_{qt}_{h}",
                                    tag="rec", bufs=2)
                    nc.vector.reciprocal(rec[:], ps_bc[:])
                    ot = work.tile([P, TC], md, name=f"outT{rep}_{qt}_{h}",
                                   tag="outT", bufs=8)
                    nc.vector.tensor_mul(ot[:], av[:], rec[:])
                    outT[h] = ot
                drain(1)

            pending.extend(make_outproj(qt, outT))
            pending.append(lambda q=qt: finish_chunk(q))

        drain(len(pending))


def _get_nc():
    name = os.environ.get("ATTN_MM_DTYPE", "float32r")
    reps = int(os.environ.get("ATTN_REPS", "1"))
    key = (name, reps)
    if key not in _CACHE:
        _CACHE[key] = _build(name, reps)
    return _CACHE[key]


last_exec_time_ns = None


def _pack_w(wT):
    # [2048, 512] -> [1024, 1024]: packed[j*128+p, sub*512+f] =
    # wT[(2j+sub)*128+p, f]
    return np.ascontiguousarray(
        wT.reshape(NJ, 2, P, F).swapaxes(1, 2).reshape(D // 2, 2 * F)
    )


def make_in_maps(x, w_qkv, w_out):
    x = np.asarray(x, dtype=np.float32)
    w_qkv = np.asarray(w_qkv, dtype=np.float32)
    w_out = np.asarray(w_out, dtype=np.float32)
    xTs = [np.ascontiguousarray(x[b].T) for b in range(B)]
    in_maps = []
    for c in range(N_CORES):
        b, hg = divmod(c, NHG)
        sl = slice(hg * F, (hg + 1) * F)
        in_maps.append({
            "xT": xTs[b],
            "wqP": _pack_w(w_qkv[0 * D:1 * D][sl].T),
            "wkP": _pack_w(w_qkv[1 * D:2 * D][sl].T),
            "wvP": _pack_w(w_qkv[2 * D:3 * D][sl].T),
            "woT": np.ascontiguousarray(w_out[:, sl].T),
        })
    return in_maps


def kernel(x, w_qkv, w_out):
    import time

    from concourse import bass_utils

    global last_exec_time_ns
    nc = _get_nc()
    in_maps = make_in_maps(x, w_qkv, w_out)

    trace = bool(int(os.environ.get("ATTN_TRACE", "0")))
    res = None
    last_err = None
    for attempt in range(3):
        try:
            res = bass_utils.run_bass_kernel_spmd(
                nc, in_maps, core_ids=list(range(N_CORES)), trace=trace
            )
            break
        except Exception as e:  # transient axon mesh desyncs
            last_err = e
            time.sleep(10 * (attempt + 1))
    if res is None:
        raise last_err
    last_exec_time_ns = res.exec_time_ns

    outs = [np.asarray(res.results[c]["out"]).astype(np.float32)
            for c in range(N_CORES)]
    # chunked RS layout: core r of a batch group holds, for chunks 0..2,
    # the summed rows qt*TC + r*128 .. +128; for the split last chunk it
    # holds rows 3*TC + hf*256 + r*64 .. +64 for hf in {0, 1}.
    RW = TC // NHG
    full = []
    for b in range(B):
        arr = np.stack(outs[b * NHG:(b + 1) * NHG])      # [r, NTC*RW, D]
        fb = np.empty((T, D), np.float32)
        head = arr[:, :(NTC - 1) * RW].reshape(NHG, NTC - 1, RW, D)
        fb[:(NTC - 1) * TC] = head.transpose(1, 0, 2, 3).reshape(-1, D)
        tail = arr[:, (NTC - 1) * RW:].reshape(NHG, 2, RW // 2, D)
        fb[(NTC - 1) * TC:] = tail.transpose(1, 0, 2, 3).reshape(-1, D)
        full.append(fb)
    return np.stack(full)


# revision 16
# speedup vs baseline: 1.6089x; 1.6089x over previous
"""Multi-head causal attention (dense transformer block) on 8 Trainium2 cores.

Sharding: 2-way data parallel over batch x 4-way tensor parallel over heads.
Core c handles batch c//4 and heads 4*(c%4) .. 4*(c%4)+3.

Per-core pipeline (all activation layouts chosen so no on-device transposes
are needed; host pre-transposes x and the weight shards once):
  1. QT/KT [hd, t] (bf16) and V [t, hd] (bf16) projections from xT [d, t]
     at fp32r. Weight tiles arrive pre-packed so one DMA carries two
     d-subtiles. Weight DMAs issue on the DVE queue, x tiles on SP, PSUM
     evictions on ACT, so no queue serializes another phase's traffic.
  2. Attention per (head, q-chunk) in bf16 with scores computed transposed
     (S^T [k, q]) at exact causal widths (bf16 matmul has no minimum
     moving-dim for full rate). exp on ACT without max-subtraction; causal
     masking via a resident upper-triangle 0/1 mask multiplied on DVE
     (keeps the Pool queue free for collectives). Softmax denominators are
     accumulated across k-tiles on DVE (bf16 2x mode) and partition-summed
     + broadcast in a single ones[128,128] matmul per (head, chunk) —
     removing the per-tile denominator matmuls from the PE critical path.
  3. Output projection at fp32r from resident weights. Its matmuls are
     interleaved into the NEXT chunk's attention steps so the PE never
     stalls while ACT works through exp.
  4. Chunked bf16 ReduceScatter over the 4 cores sharing a batch; host
     casts to f32 and concatenates the row shards.
"""

import os
import sys

sys.path.insert(0, "/opt/trn_rl_repo")

import numpy as np

N_CORES = 8
B = 2
T = 2048          # sequence length
D = 2048          # model dim
P = 128           # partitions
HD = 128          # head dim
NHG = 4           # head-groups (cores per batch)
HPC = 4           # heads per core
F = HPC * HD      # 512 per-core q/k/v feature width
TC = 512          # token chunk (matmul free dim)
NTC = T // TC     # 4 token chunks
ND = D // P       # 16 d-subtiles
NJ = ND // 2      # 8 packed weight tiles (2 d-subtiles each)
SCALE = float(HD) ** -0.5

_CACHE = {}


def _build(mm_dtype_name: str, reps: int = 1, with_rs: bool = True):
    import concourse.bacc as bacc
    import concourse.mybir as mybir
    import concourse.tile as tile

    dt = mybir.dt
    f32 = dt.float32
    bf = dt.bfloat16
    md = getattr(dt, mm_dtype_name)  # dtype of fp32-path PE-input tiles

    nc = bacc.Bacc(
        "TRN2", target_bir_lowering=False, debug=False, num_devices=N_CORES
    )

    xT = nc.dram_tensor("xT", [D, T], md, kind="ExternalInput")
    # packed: [j*128+p, sub*512+f] = W^T[(2j+sub)*128+p, f]
    wqP = nc.dram_tensor("wqP", [D // 2, 2 * F], md, kind="ExternalInput")
    wkP = nc.dram_tensor("wkP", [D // 2, 2 * F], md, kind="ExternalInput")
    wvP = nc.dram_tensor("wvP", [D // 2, 2 * F], md, kind="ExternalInput")
    woT = nc.dram_tensor("woT", [F, D], md, kind="ExternalInput")
    out = nc.dram_tensor("out", [T // NHG, D], bf, kind="ExternalOutput")

    with nc.allow_low_precision(reason="bf16 attention internals"), \
         tile.TileContext(nc) as tc:
        with (
            tc.tile_pool(name="const", bufs=1) as const,
            tc.tile_pool(name="resident", bufs=1) as res_pool,
            tc.tile_pool(name="dram", bufs=1, space="DRAM") as dram,
        ):
            ones128 = const.tile([P, P], bf)
            nc.vector.memset(ones128[:], 1.0)
            # upper-triangle keep-mask for the diagonal score blocks:
            # tri[p, c] = 1 if c >= p else 0  (S^T layout: p=k, c=q)
            tri = const.tile([P, P], bf)
            nc.vector.memset(tri[:], 1.0)
            nc.gpsimd.affine_select(
                tri[:], tri[:],
                pattern=[[1, P]],
                compare_op=mybir.AluOpType.is_ge,
                fill=0.0,
                base=0,
                channel_multiplier=-1,
            )

            # ---- resident activation buffers ----
            QT = [res_pool.tile([P, T], bf, name=f"QT{h}") for h in range(HPC)]
            KT = [res_pool.tile([P, T], bf, name=f"KT{h}") for h in range(HPC)]
            V = [res_pool.tile([P, F], bf, name=f"V{i}") for i in range(T // P)]
            WO = [
                [res_pool.tile([P, 2 * TC], md, name=f"WO{ci}_{etp}")
                 for etp in range(NTC // 2)]
                for ci in range(HPC)
            ]

            bounce = [dram.tile([TC, D], bf, name=f"bounce{qt}")
                      for qt in range(NTC - 1)]
            bounce += [dram.tile([TC // 2, D], bf, name=f"bounce3{hf}")
                       for hf in range(2)]
            rs_out = [dram.tile([TC // NHG, D], bf, name=f"rs_out{qt}")
                      for qt in range(NTC - 1)]
            rs_out += [dram.tile([TC // 2 // NHG, D], bf, name=f"rs_out3{hf}")
                       for hf in range(2)]

            for rep in range(reps):
                _build_body(nc, tc, mybir, md, bf, f32, rep,
                            xT, wqP, wkP, wvP, woT, out,
                            ones128, tri, QT, KT, V, WO,
                            bounce, rs_out, with_rs)

    nc.compile()
    return nc


def _build_body(nc, tc, mybir, md, bf, f32, rep,
                xT, wqP, wkP, wvP, woT, out,
                ones128, tri, QT, KT, V, WO,
                bounce, rs_out, with_rs=True):
    # ---- phase 0: resident output-projection weights (Pool/SWDGE queue,
    # idle during phase 1, so these don't delay the first wq tiles) ----
    if rep == 0:
        for ci in range(HPC):
            for etp in range(NTC // 2):
                nc.gpsimd.dma_start(
                    WO[ci][etp][:],
                    woT.ap()[ci * P:(ci + 1) * P,
                             etp * 2 * TC:(etp + 1) * 2 * TC],
                )

    # ---- phase 1: projections ----
    # Two supersteps of 1024 tokens; each loads the packed q/k/v weights
    # once (24 MB instead of 48 MB of weight traffic per pass over x).
    TG = 2 * TC
    with tc.tile_pool(name=f"psum1_{rep}", bufs=1, space="PSUM") as psum1, \
         tc.tile_pool(name=f"xw_{rep}", bufs=3) as xw_pool:
        for tg in range(T // TG):
            xts = []
            for di in range(ND):
                xt = xw_pool.tile(
                    [P, TG], md, name=f"xt_{rep}_{tg}_{di}", tag="xt",
                    bufs=ND + 2,
                )
                nc.sync.dma_start(
                    xt[:],
                    xT.ap()[di * P:(di + 1) * P, tg * TG:(tg + 1) * TG],
                )
                xts.append(xt)
            wts = {}
            for wname, wP in (("q", wqP), ("k", wkP), ("v", wvP)):
                for j in range(NJ):
                    wt = xw_pool.tile(
                        [P, 2 * F], md, name=f"w{wname}_{rep}_{tg}_{j}",
                        tag="wt", bufs=6,
                    )
                    nc.scalar.dma_start(wt[:], wP.ap()[j * P:(j + 1) * P, :])
                    wts[wname, j] = wt
            for wname, dest in (("q", QT), ("k", KT)):
                pss = [
                    psum1.tile(
                        [P, TC], f32, name=f"ps_{wname}{h}{th}_{rep}_{tg}",
                        tag="pq", bufs=8,
                    )
                    for h in range(HPC) for th in range(2)
                ]
                for j in range(NJ):
                    wt = wts[wname, j]
                    for sub in range(2):
                        di = 2 * j + sub
                        for h in range(HPC):
                            for th in range(2):
                                nc.tensor.matmul(
                                    pss[2 * h + th][:],
                                    wt[:, sub * F + h * HD:
                                       sub * F + (h + 1) * HD],
                                    xts[di][:, th * TC:(th + 1) * TC],
                                    start=(di == 0),
                                    stop=(di == ND - 1),
                                )
                for h in range(HPC):
                    for th in range(2):
                        col = tg * TG + th * TC
                        nc.vector.tensor_copy(
                            dest[h][:, col:col + TC], pss[2 * h + th][:]
                        )
            pss = [
                psum1.tile(
                    [P, F], f32, name=f"ps_v{ts}_{rep}_{tg}", tag="pq", bufs=8
                )
                for ts in range(TG // P)
            ]
            for j in range(NJ):
                wt = wts["v", j]
                for sub in range(2):
                    di = 2 * j + sub
                    for ts in range(TG // P):
                        nc.tensor.matmul(
                            pss[ts][:],
                            xts[di][:, ts * P:(ts + 1) * P],
                            wt[:, sub * F:(sub + 1) * F],
                            start=(di == 0),
                            stop=(di == ND - 1),
                        )
            # V evictions on ACT (GPSIMD cannot read PSUM) so the DVE
            # queue is clear for the first attention masks/accumulates
            for ts in range(TG // P):
                nc.scalar.copy(V[tg * (TG // P) + ts][:], pss[ts][:])

    # ---- phases 2+3, software-pipelined per q chunk ----
    with tc.tile_pool(name=f"psum2_{rep}", bufs=1, space="PSUM") as psum2, \
         tc.tile_pool(name=f"work_{rep}", bufs=6) as work:

        def make_outproj(qt, outT):
            """Closures, one per (ts, etp): 8 fp32r matmuls + 2 evictions
            (split ACT/DVE) + the bounce-row DMA (SP)."""
            last = qt == NTC - 1
            closures = []

            def mk(ts, etp):
                def run():
                    fin = work.tile(
                        [P, 2 * TC], bf, name=f"fin{rep}_{qt}_{ts}_{etp}",
                        tag="fin", bufs=4,
                    )
                    # the last chunk's closures run after attention is done,
                    # so they may borrow the idle 'st' banks for deeper
                    # pipelining of the eviction chain
                    tag = "f" if (not last or (ts + etp) % 2 == 0) else "st"
                    psf = [
                        psum2.tile(
                            [P, TC], f32,
                            name=f"ps_f{rep}_{qt}_{ts}_{etp}_{ee}",
                            tag=tag, bufs=2,
                        )
                        for ee in range(2)
                    ]
                    for ci in range(HPC):
                        for ee in range(2):
                            nc.tensor.matmul(
                                psf[ee][:],
                                outT[ci][:, ts * P:(ts + 1) * P],
                                WO[ci][etp][:, ee * TC:(ee + 1) * TC],
                                start=(ci == 0),
                                stop=(ci == HPC - 1),
                            )
                    nc.scalar.copy(fin[:, 0:TC], psf[0][:])
                    nc.vector.tensor_copy(fin[:, TC:2 * TC], psf[1][:])
                    if last:
                        dst = bounce[NTC - 1 + ts // 2]
                        drow = (ts % 2) * P
                    else:
                        dst = bounce[qt]
                        drow = ts * P
                    nc.sync.dma_start(
                        dst[drow:drow + P,
                            etp * 2 * TC:(etp + 1) * 2 * TC],
                        fin[:],
                    )
                return run

            for ts in range(TC // P):
                for etp in range(NTC // 2):
                    closures.append(mk(ts, etp))
            return closures

        def finish_chunk(qt):
            """Post-outproj collective for chunk qt."""
            if qt < NTC - 1:
                if with_rs:
                    nc.gpsimd.collective_compute(
                        "ReduceScatter",
                        mybir.AluOpType.add,
                        replica_groups=[[0, 1, 2, 3], [4, 5, 6, 7]],
                        ins=[bounce[qt].opt()],
                        outs=[rs_out[qt].opt()],
                    )
                    nc.gpsimd.dma_start(
                        out.ap()[qt * (TC // NHG):(qt + 1) * (TC // NHG), :],
                        rs_out[qt][:],
                    )
                else:
                    nc.gpsimd.dma_start(
                        out.ap()[qt * (TC // NHG):(qt + 1) * (TC // NHG), :],
                        bounce[qt][0:TC // NHG, :],
                    )
            else:
                for hf in range(2):
                    rw = TC // 2 // NHG
                    base = qt * (TC // NHG) + hf * rw
                    if with_rs:
                        nc.gpsimd.collective_compute(
                            "ReduceScatter",
                            mybir.AluOpType.add,
                            replica_groups=[[0, 1, 2, 3], [4, 5, 6, 7]],
                            ins=[bounce[NTC - 1 + hf].opt()],
                            outs=[rs_out[NTC - 1 + hf].opt()],
                        )
                        nc.gpsimd.dma_start(
                            out.ap()[base:base + rw, :],
                            rs_out[NTC - 1 + hf][:],
                        )
                    else:
                        nc.gpsimd.dma_start(
                            out.ap()[base:base + rw, :],
                            bounce[NTC - 1 + hf][0:rw, :],
                        )

        pending = []      # rolling queue of outproj closures (FIFO)

        def drain(n):
            for _ in range(min(n, len(pending))):
                pending.pop(0)()

        for qt in range(NTC):
            outT = {}
            n_k = (qt + 1) * (TC // P)  # causal: k-subtiles needed
            diag0 = qt * (TC // P)
            # wide (full-512) off-diagonal tiles first, narrow diagonal
            # tiles last: the pipeline-drain exps at each head-pair
            # boundary are then the cheap narrow ones. The first entry is
            # always full-width (off-diag, or dj=0 when qt==0), which the
            # start=True AV matmul and the acc-init copy both rely on.
            korder = list(range(diag0)) + list(range(diag0, n_k))
            if qt == 0:
                korder = list(range(n_k))
            SKEW = 2
            for hp in (0, 2):  # head pairs, emission interleaved
                heads = (hp, hp + 1)
                ps_out = {
                    h: psum2.tile(
                        [P, TC], f32, name=f"ps_out{rep}_{qt}_{h}",
                        tag="out", bufs=2,
                    )
                    for h in heads
                }
                acc = {
                    h: work.tile(
                        [P, TC], bf, name=f"acc{rep}_{qt}_{h}", tag="acc",
                        bufs=4,
                    )
                    for h in heads
                }
                pts = {}

                def live0(kt):
                    # exact causal width: bf16 matmuls run full rate at any
                    # moving size, so no 256-wide clamp is needed
                    return max(0, kt - diag0) * P

                for step in range(n_k + SKEW):
                    if step < n_k:
                        kt = korder[step]
                        c0 = live0(kt)
                        for h in heads:
                            ps_st = psum2.tile(
                                [P, TC], f32,
                                name=f"ps_st{rep}_{qt}_{h}_{kt}",
                                tag="st", bufs=2,
                            )
                            nc.tensor.matmul(
                                ps_st[:, c0:],
                                KT[h][:, kt * P:(kt + 1) * P],
                                QT[h][:, qt * TC + c0:(qt + 1) * TC],
                                start=True,
                                stop=True,
                            )
                            pt = work.tile(
                                [P, TC], bf, name=f"pt{rep}_{qt}_{h}_{kt}",
                                tag="pt", bufs=8,
                            )
                            nc.scalar.activation(
                                pt[:, c0:], ps_st[:, c0:],
                                mybir.ActivationFunctionType.Exp,
                                scale=SCALE,
                            )
                            dj = kt - diag0
                            if dj >= 0:
                                # zero the strictly-upper part of the
                                # diagonal 128x128 block (q < k)
                                me = (dj + 1) * P
                                nc.vector.tensor_mul(
                                    pt[:, c0:me], pt[:, c0:me], tri[:]
                                )
                            if step == 0:
                                nc.vector.tensor_copy(acc[h][:], pt[:])
                            else:
                                nc.vector.tensor_add(
                                    acc[h][:, c0:], acc[h][:, c0:],
                                    pt[:, c0:],
                                )
                            pts[h, kt] = pt
                    if step >= SKEW:
                        idx = step - SKEW
                        k = korder[idx]
                        c0 = live0(k)
                        for h in heads:
                            nc.tensor.matmul(
                                ps_out[h][:, c0:],
                                V[k][:, h * HD:(h + 1) * HD],
                                pts[h, k][:, c0:],
                                start=(idx == 0),
                                stop=(idx == n_k - 1),
                            )
                    drain(1)
                for h in heads:
                    ps_bc = psum2.tile(
                        [P, TC], f32, name=f"ps_bc{rep}_{qt}_{h}", tag="aux",
                        bufs=2,
                    )
                    nc.tensor.matmul(
                        ps_bc[:], ones128[:], acc[h][:],
                        start=True, stop=True,
                    )
                    # evict AV to SBUF on ACT right away: frees the PSUM
                    # bank for the next chunk without waiting on the DVE
                    # reciprocal chain
                    av = work.tile([P, TC], f32, name=f"av{rep}_{qt}_{h}",
                                   tag="av", bufs=2)
                    nc.scalar.copy(av[:], ps_out[h][:])
                    rec = work.tile([P, TC], f32, name=f"rec{rep}_{qt}_{h}",
                                    tag="rec", bufs=2)
                    nc.vector.reciprocal(rec[:], ps_bc[:])
                    ot = work.tile([P, TC], md, name=f"outT{rep}_{qt}_{h}",
                                   tag="outT", bufs=8)
                    nc.vector.tensor_mul(ot[:], av[:], rec[:])
                    outT[h] = ot
                drain(1)

            pending.extend(make_outproj(qt, outT))
            pending.append(lambda q=qt: finish_chunk(q))

        drain(len(pending))


def _get_nc():
    name = os.environ.get("ATTN_MM_DTYPE", "float32r")
    reps = int(os.environ.get("ATTN_REPS", "1"))
    key = (name, reps)
    if key not in _CACHE:
        _CACHE[key] = _build(name, reps)
    return _CACHE[key]


last_exec_time_ns = None


def _pack_w(wT):
    # [2048, 512] -> [1024, 1024]: packed[j*128+p, sub*512+f] =
    # wT[(2j+sub)*128+p, f]
    return np.ascontiguousarray(
        wT.reshape(NJ, 2, P, F).swapaxes(1, 2).reshape(D // 2, 2 * F)
    )


def make_in_maps(x, w_qkv, w_out):
    x = np.asarray(x, dtype=np.float32)
    w_qkv = np.asarray(w_qkv, dtype=np.float32)
    w_out = np.asarray(w_out, dtype=np.float32)
    xTs = [np.ascontiguousarray(x[b].T) for b in range(B)]
    in_maps = []
    for c in range(N_CORES):
        b, hg = divmod(c, NHG)
        sl = slice(hg * F, (hg + 1) * F)
        in_maps.append({
            "xT": xTs[b],
            "wqP": _pack_w(w_qkv[0 * D:1 * D][sl].T),
            "wkP": _pack_w(w_qkv[1 * D:2 * D][sl].T),
            "wvP": _pack_w(w_qkv[2 * D:3 * D][sl].T),
            "woT": np.ascontiguousarray(w_out[:, sl].T),
        })
    return in_maps


def kernel(x, w_qkv, w_out):
    import time

    from concourse import bass_utils

    global last_exec_time_ns
    nc = _get_nc()
    in_maps = make_in_maps(x, w_qkv, w_out)

    trace = bool(int(os.environ.get("ATTN_TRACE", "0")))
    res = None
    last_err = None
    for attempt in range(3):
        try:
            res = bass_utils.run_bass_kernel_spmd(
                nc, in_maps, core_ids=list(range(N_CORES)), trace=trace
            )
            break
        except Exception as e:  # transient axon mesh desyncs
            last_err = e
            time.sleep(10 * (attempt + 1))
    if res is None:
        raise last_err
    last_exec_time_ns = res.exec_time_ns

    outs = [np.asarray(res.results[c]["out"]).astype(np.float32)
            for c in range(N_CORES)]
    # chunked RS layout: core r of a batch group holds, for chunks 0..2,
    # the summed rows qt*TC + r*128 .. +128; for the split last chunk it
    # holds rows 3*TC + hf*256 + r*64 .. +64 for hf in {0, 1}.
    RW = TC // NHG
    full = []
    for b in range(B):
        arr = np.stack(outs[b * NHG:(b + 1) * NHG])      # [r, NTC*RW, D]
        fb = np.empty((T, D), np.float32)
        head = arr[:, :(NTC - 1) * RW].reshape(NHG, NTC - 1, RW, D)
        fb[:(NTC - 1) * TC] = head.transpose(1, 0, 2, 3).reshape(-1, D)
        tail = arr[:, (NTC - 1) * RW:].reshape(NHG, 2, RW // 2, D)
        fb[(NTC - 1) * TC:] = tail.transpose(1, 0, 2, 3).reshape(-1, D)
        full.append(fb)
    return np.stack(full)
